# revision 29
# baseline (speedup 1.0000x reference)
"""LIF spike recurrence kernel for Trainium2 (8 NeuronCores, SPMD).

Problem: x [32, 128, 32, 32, 8] f32, recurrence over last (time) dim:
    u_t = TAU * u_{t-1} * (1 - o_{t-1}) + x_t
    o_t = 1[u_t - VTH > 0]
Output: o [32, 128, 32, 32, 8] f32 (0.0 / 1.0 spikes).

v3 design (vs the v2 time-interleaved layout, 131.8 us):
  - Host transposes each core's shard to TIME-MAJOR [128, T, 4096] so every
    device-side op is contiguous (v2 lost ~2x engine throughput to
    stride-32B access).
  - Spikes leave the device as uint8 {0,1} (4 MB instead of 16 MB per core
    of output traffic); host casts back to f32.
  - The recurrence (2 two-input STT ops per step) runs entirely on DVE:
    fp32 two-input ops are port-limited to 1 elem/cyc there, and GpSimd
    cannot help (it steals one of DVE's two SBUF ports, so DVE+GpSimd
    combined throughput == DVE alone; measured). TensorE identity-matmul
    accumulation was also measured slower (fp32 matmuls decompose into
    2 MATMUL instructions, ~5.5 cyc/col). 2 big tiles of 2048 columns
    minimize the ~151-cycle fixed cost per DVE op.
  - Thresholds run on ACT (concurrent with DVE, no port conflict), one
    Sign per time-plane in completion order: u8(Sign(u - VTH)) saturates
    {-1,0,1} to {0,0,1} == (u > VTH), verified exact on HW. The very
    last plane is thresholded on DVE itself (is_gt, 2x mode) in halves
    to shorten the tail.
  - Input DMAs are per time-plane on the sync HWDGE ring; output DMAs
    ride the scalar HWDGE ring, one [F]-byte block per plane.

Exactness: mask = (u <= VTH) in {0,1}; c = mask*u (or u*(mask*TAU));
u' = c*TAU + x_t reproduces TAU*u*(1-o)+x_t bitwise (mults by {0,1} and
2^-2 exact), same argument as the proven v2 kernel.
"""

import numpy as np

TAU = 0.25
VTH = 0.3
N_CORES = 8
P = 128
T = 8
B_LOC = 4  # batches per core
NPP = B_LOC * 128 * 32 * 32 // P  # 4096 pixel-columns per partition

# (pixel-columns, engine) per tile; 'v' = DVE. (A GpSimd variant was
# measured and removed: GpSimd steals one of DVE's two SBUF ports, so any
# concurrent GpSimd streaming makes the split a wash or worse, and its
# 2-ALU tensor_scalar path runs a pathological ~19 cyc/elem.)
TILES = [
    (2048, "v"),
    (2048, "v"),
]
assert sum(f for f, _ in TILES) == NPP

_CACHE = {}


def _cost_model():
    """Per-(tile, step) estimated completion times, to order ACT work."""
    eng_clock = {"v": 0.0}
    done = {}  # (tile_idx, t) -> est finish ns of u_t
    for k, (fi, e) in enumerate(TILES):
        # crude DMA gate: plane 0 of tile k arrives roughly in stream order
        # (~0.34 ns per byte-per-partition at ~350 GB/s across 128 parts).
        arr0 = 2500.0 + sum(f for f, _ in TILES[:k]) * T * 4 * 0.34
        clk = max(eng_clock[e], arr0)
        done[(k, 0)] = clk
        for t in range(1, T):
            # measured: DVE fp32 2-input STT = (FD + 151) cyc @ 0.96 GHz
            clk += 2 * (fi + 151) / 0.96
            done[(k, t)] = clk
        eng_clock[e] = clk
    return done


def _build_nc():
    import concourse.tile as tile
    from concourse import bacc, mybir

    f32 = mybir.dt.float32
    u8 = mybir.dt.uint8
    Alu = mybir.AluOpType
    AF = mybir.ActivationFunctionType

    nc = bacc.Bacc(
        "TRN2",
        target_bir_lowering=False,
        debug=False,
        enable_asserts=False,
        use_seq_codegen=True,
        num_devices=N_CORES,
    )
    # Time-major input [P, T, NPP]; output = per-tile [T, F] blocks packed
    # back-to-back per partition (byte offset T*off for the tile at off).
    x_d = nc.dram_tensor("x", [P, T, NPP], f32, kind="ExternalInput").ap()
    o_d = nc.dram_tensor("o", [P, T * NPP], u8, kind="ExternalOutput").ap()

    # ACT activation bias needs a pre-registered const AP.
    cb = nc.alloc_sbuf_tensor("const-f32-negvth", [128, 1], f32)
    nc.gpsimd.memset(cb.ap(), -VTH)
    nc.const_aps.aps[(f32, -VTH)] = cb.ap()
    nc.all_engine_barrier()

    offs = []
    o = 0
    for fi, _ in TILES:
        offs.append(o)
        o += fi

    done = _cost_model()

    bv = min(2, len(TILES))
    with tile.TileContext(nc) as tc:
        with tc.tile_pool(name="vp", bufs=bv) as vp, tc.tile_pool(
            name="ov", bufs=bv
        ) as ovp, tc.tile_pool(name="cv", bufs=2) as cvp:
            xts, ots = {}, {}
            for k, (fi, e) in enumerate(TILES):
                xts[k] = vp.tile([P, T, fi], f32, tag="xt", name=f"xt{k}")
                ots[k] = ovp.tile([P, T * fi], u8, tag="ot", name=f"ot{k}")

            # Input DMAs: per time-plane, in tile/plane order on the sync
            # HWDGE ring (plane t gates only step t; experiments splitting
            # or dual-ringing the head planes lost to DMA cold-ramp).
            def in_dma(k, t):
                fi, off = TILES[k][0], offs[k]
                nc.sync.dma_start(xts[k][:, t, :], x_d[:, t, off : off + fi])

            for k in range(len(TILES)):
                for t in range(T):
                    in_dma(k, t)

            # Recurrence on DVE, in place over the x planes.
            for k, (fi, e) in enumerate(TILES):
                xt = xts[k]
                for t in range(1, T):
                    c = cvp.tile([P, fi], f32, tag="c")
                    up = xt[:, t - 1, :]
                    # c = (u_{t-1} <= VTH) * u_{t-1}
                    nc.vector.scalar_tensor_tensor(
                        c[:], up, VTH, up, op0=Alu.is_le, op1=Alu.mult
                    )
                    # u_t = c * TAU + x_t   (in place)
                    nc.vector.scalar_tensor_tensor(
                        xt[:, t, :], c[:], TAU, xt[:, t, :],
                        op0=Alu.mult, op1=Alu.add,
                    )

            # Thresholds on ACT, per plane, in estimated readiness order;
            # the tile's output DMA follows its last plane.
            events = sorted(
                ((done[(k, t)], k, t) for k, (fi, _) in enumerate(TILES)
                 for t in range(T)),
            )
            for ei, (_, k, t) in enumerate(events):
                fi, off = TILES[k][0], offs[k]
                if ei == len(events) - 1:
                    # Tail: threshold the last plane on DVE right after its
                    # own final STT (no cross-engine hop), in halves so the
                    # output DMA starts before the second half finishes.
                    h = fi // 2
                    for s in range(2):
                        sl = slice(t * fi + s * h, t * fi + (s + 1) * h)
                        nc.vector.tensor_scalar(
                            ots[k][:, sl], xts[k][:, t, s * h : (s + 1) * h],
                            VTH, None, op0=Alu.is_gt,
                        )
                        nc.scalar.dma_start(
                            o_d[:, T * off + sl.start : T * off + sl.stop],
                            ots[k][:, sl],
                        )
                    continue
                nc.scalar.activation(
                    ots[k][:, t * fi : (t + 1) * fi], xts[k][:, t, :],
                    AF.Sign, bias=-VTH,
                )
                # Per-plane output block, contiguous fi bytes per partition.
                nc.scalar.dma_start(
                    o_d[:, T * off + t * fi : T * off + (t + 1) * fi],
                    ots[k][:, t * fi : (t + 1) * fi],
                )
    nc.compile()
    return nc


def _get_nc():
    if "nc" not in _CACHE:
        _CACHE["nc"] = _build_nc()
    return _CACHE["nc"]


def _shard(x: np.ndarray):
    xs = np.ascontiguousarray(x, dtype=np.float32)
    shards = []
    for i in range(N_CORES):
        s = xs[i * B_LOC : (i + 1) * B_LOC].reshape(P, NPP, T)
        shards.append(np.ascontiguousarray(s.transpose(0, 2, 1)))  # [P,T,NPP]
    return shards


def _unshard_one(o_flat: np.ndarray) -> np.ndarray:
    # o_flat u8 [P, T*NPP] of per-tile [T, F] blocks -> [B_LOC,128,32,32,T]
    o_px = np.empty((P, NPP, T), dtype=np.uint8)
    off = 0
    for fi, _ in TILES:
        blk = o_flat[:, T * off : T * (off + fi)].reshape(P, T, fi)
        o_px[:, off : off + fi, :] = blk.transpose(0, 2, 1)
        off += fi
    return o_px.reshape(B_LOC, 128, 32, 32, T)


def _run(in_maps, **kwargs):
    from concourse.bass_utils import run_bass_kernel_spmd

    nc = _get_nc()
    return run_bass_kernel_spmd(nc, in_maps, core_ids=list(range(N_CORES)), **kwargs)


def kernel(x: np.ndarray) -> np.ndarray:
    in_maps = [{"x": s} for s in _shard(x)]
    res = _run(in_maps)
    outs = [_unshard_one(res.results[i]["o"]) for i in range(N_CORES)]
    return np.concatenate(outs, axis=0).astype(np.float32)


# revision 30
# speedup vs baseline: 1.0239x; 1.0239x over previous
"""LIF spike recurrence kernel for Trainium2 (8 NeuronCores, SPMD).

Problem: x [32, 128, 32, 32, 8] f32, recurrence over last (time) dim:
    u_t = TAU * u_{t-1} * (1 - o_{t-1}) + x_t
    o_t = 1[u_t - VTH > 0]
Output: o [32, 128, 32, 32, 8] f32 (0.0 / 1.0 spikes).

v3 design (vs the v2 time-interleaved layout, 131.8 us):
  - Host transposes each core's shard to TIME-MAJOR [128, T, 4096] so every
    device-side op is contiguous (v2 lost ~2x engine throughput to
    stride-32B access).
  - Spikes leave the device as uint8 {0,1} (4 MB instead of 16 MB per core
    of output traffic); host casts back to f32.
  - The recurrence (2 two-input STT ops per step) runs entirely on DVE:
    fp32 two-input ops are port-limited to 1 elem/cyc there, and GpSimd
    cannot help (it steals one of DVE's two SBUF ports, so DVE+GpSimd
    combined throughput == DVE alone; measured). TensorE identity-matmul
    accumulation was also measured slower (fp32 matmuls decompose into
    2 MATMUL instructions, ~5.5 cyc/col). 2 big tiles of 2048 columns
    minimize the ~151-cycle fixed cost per DVE op.
  - Thresholds run on ACT (concurrent with DVE, no port conflict), one
    Sign per time-plane in completion order: u8(Sign(u - VTH)) saturates
    {-1,0,1} to {0,0,1} == (u > VTH), verified exact on HW. The very
    last plane is thresholded on DVE itself (is_gt, 2x mode) in halves
    to shorten the tail.
  - Input DMAs are per time-plane on the sync HWDGE ring; output DMAs
    ride the scalar HWDGE ring, one [F]-byte block per plane.

Exactness: mask = (u <= VTH) in {0,1}; c = mask*u (or u*(mask*TAU));
u' = c*TAU + x_t reproduces TAU*u*(1-o)+x_t bitwise (mults by {0,1} and
2^-2 exact), same argument as the proven v2 kernel.
"""

import numpy as np

TAU = 0.25
VTH = 0.3
N_CORES = 8
P = 128
T = 8
B_LOC = 4  # batches per core
NPP = B_LOC * 128 * 32 * 32 // P  # 4096 pixel-columns per partition

# (pixel-columns, engine) per tile; 'v' = DVE. (A GpSimd variant was
# measured and removed: GpSimd steals one of DVE's two SBUF ports, so any
# concurrent GpSimd streaming makes the split a wash or worse, and its
# 2-ALU tensor_scalar path runs a pathological ~19 cyc/elem.)
TILES = [
    (2048, "v"),
    (2048, "v"),
]
assert sum(f for f, _ in TILES) == NPP

_CACHE = {}


def _cost_model():
    """Per-(tile, step) estimated completion times, to order ACT work."""
    eng_clock = {"v": 0.0}
    done = {}  # (tile_idx, t) -> est finish ns of u_t
    for k, (fi, e) in enumerate(TILES):
        # crude DMA gate: plane 0 of tile k arrives roughly in stream order
        # (~0.34 ns per byte-per-partition at ~350 GB/s across 128 parts).
        arr0 = 2500.0 + sum(f for f, _ in TILES[:k]) * T * 4 * 0.34
        clk = max(eng_clock[e], arr0)
        done[(k, 0)] = clk
        for t in range(1, T):
            # measured: DVE fp32 2-input STT = (FD + 151) cyc @ 0.96 GHz
            clk += 2 * (fi + 151) / 0.96
            done[(k, t)] = clk
        eng_clock[e] = clk
    return done


def _build_nc():
    import concourse.tile as tile
    from concourse import bacc, mybir

    f32 = mybir.dt.float32
    u8 = mybir.dt.uint8
    Alu = mybir.AluOpType
    AF = mybir.ActivationFunctionType

    nc = bacc.Bacc(
        "TRN2",
        target_bir_lowering=False,
        debug=False,
        enable_asserts=False,
        num_devices=N_CORES,
    )
    # Time-major input [P, T, NPP]; output = per-tile [T, F] blocks packed
    # back-to-back per partition (byte offset T*off for the tile at off).
    x_d = nc.dram_tensor("x", [P, T, NPP], f32, kind="ExternalInput").ap()
    o_d = nc.dram_tensor("o", [P, T * NPP], u8, kind="ExternalOutput").ap()

    # ACT activation bias needs a pre-registered const AP.
    cb = nc.alloc_sbuf_tensor("const-f32-negvth", [128, 1], f32)
    nc.gpsimd.memset(cb.ap(), -VTH)
    nc.const_aps.aps[(f32, -VTH)] = cb.ap()
    nc.all_engine_barrier()

    offs = []
    o = 0
    for fi, _ in TILES:
        offs.append(o)
        o += fi

    done = _cost_model()

    bv = min(2, len(TILES))
    with tile.TileContext(nc) as tc:
        with tc.tile_pool(name="vp", bufs=bv) as vp, tc.tile_pool(
            name="ov", bufs=bv
        ) as ovp, tc.tile_pool(name="cv", bufs=2) as cvp:
            xts, ots = {}, {}
            for k, (fi, e) in enumerate(TILES):
                xts[k] = vp.tile([P, T, fi], f32, tag="xt", name=f"xt{k}")
                ots[k] = ovp.tile([P, T * fi], u8, tag="ot", name=f"ot{k}")

            # Input DMAs: per time-plane, in tile/plane order on the sync
            # HWDGE ring (plane t gates only step t; experiments splitting
            # or dual-ringing the head planes lost to DMA cold-ramp).
            def in_dma(k, t):
                fi, off = TILES[k][0], offs[k]
                nc.sync.dma_start(xts[k][:, t, :], x_d[:, t, off : off + fi])

            for k in range(len(TILES)):
                for t in range(T):
                    in_dma(k, t)

            # Recurrence on DVE, in place over the x planes.
            for k, (fi, e) in enumerate(TILES):
                xt = xts[k]
                for t in range(1, T):
                    c = cvp.tile([P, fi], f32, tag="c")
                    up = xt[:, t - 1, :]
                    # c = (u_{t-1} <= VTH) * u_{t-1}
                    nc.vector.scalar_tensor_tensor(
                        c[:], up, VTH, up, op0=Alu.is_le, op1=Alu.mult
                    )
                    # u_t = c * TAU + x_t   (in place)
                    nc.vector.scalar_tensor_tensor(
                        xt[:, t, :], c[:], TAU, xt[:, t, :],
                        op0=Alu.mult, op1=Alu.add,
                    )

            # Thresholds on ACT, per plane, in estimated readiness order;
            # the tile's output DMA follows its last plane.
            events = sorted(
                ((done[(k, t)], k, t) for k, (fi, _) in enumerate(TILES)
                 for t in range(T)),
            )
            for ei, (_, k, t) in enumerate(events):
                fi, off = TILES[k][0], offs[k]
                if ei == len(events) - 1:
                    # Tail: threshold the last plane on DVE right after its
                    # own final STT (no cross-engine hop), in halves so the
                    # output DMA starts before the second half finishes.
                    h = fi // 2
                    for s in range(2):
                        sl = slice(t * fi + s * h, t * fi + (s + 1) * h)
                        nc.vector.tensor_scalar(
                            ots[k][:, sl], xts[k][:, t, s * h : (s + 1) * h],
                            VTH, None, op0=Alu.is_gt,
                        )
                        nc.scalar.dma_start(
                            o_d[:, T * off + sl.start : T * off + sl.stop],
                            ots[k][:, sl],
                        )
                    continue
                nc.scalar.activation(
                    ots[k][:, t * fi : (t + 1) * fi], xts[k][:, t, :],
                    AF.Sign, bias=-VTH,
                )
                # Per-plane output block, contiguous fi bytes per partition.
                nc.scalar.dma_start(
                    o_d[:, T * off + t * fi : T * off + (t + 1) * fi],
                    ots[k][:, t * fi : (t + 1) * fi],
                )
    nc.compile()
    return nc


def _get_nc():
    if "nc" not in _CACHE:
        _CACHE["nc"] = _build_nc()
    return _CACHE["nc"]


def _shard(x: np.ndarray):
    xs = np.ascontiguousarray(x, dtype=np.float32)
    shards = []
    for i in range(N_CORES):
        s = xs[i * B_LOC : (i + 1) * B_LOC].reshape(P, NPP, T)
        shards.append(np.ascontiguousarray(s.transpose(0, 2, 1)))  # [P,T,NPP]
    return shards


def _unshard_one(o_flat: np.ndarray) -> np.ndarray:
    # o_flat u8 [P, T*NPP] of per-tile [T, F] blocks -> [B_LOC,128,32,32,T]
    o_px = np.empty((P, NPP, T), dtype=np.uint8)
    off = 0
    for fi, _ in TILES:
        blk = o_flat[:, T * off : T * (off + fi)].reshape(P, T, fi)
        o_px[:, off : off + fi, :] = blk.transpose(0, 2, 1)
        off += fi
    return o_px.reshape(B_LOC, 128, 32, 32, T)


def _run(in_maps, **kwargs):
    from concourse.bass_utils import run_bass_kernel_spmd

    nc = _get_nc()
    return run_bass_kernel_spmd(nc, in_maps, core_ids=list(range(N_CORES)), **kwargs)


def kernel(x: np.ndarray) -> np.ndarray:
    in_maps = [{"x": s} for s in _shard(x)]
    res = _run(in_maps)
    outs = [_unshard_one(res.results[i]["o"]) for i in range(N_CORES)]
    return np.concatenate(outs, axis=0).astype(np.float32)
